# revision 1
# baseline (speedup 1.0000x reference)
"""Trainium2 Bass kernel for 16-head cross attention, tensor-parallel over 8 cores.

Reference computation (fp32):
    q = (x @ Wq).reshape(n, 16, 64)   # x [2048, 1024], Wq [1024, 1024]
    k = (ctx @ Wk).reshape(m, 16, 64) # ctx [2048, 768]
    v = (ctx @ Wv).reshape(m, 16, 64)
    out[h] = softmax(q[h] @ k[h].T / 8) @ v[h]
    y = out.reshape(n, 1024) @ Wo
Sharding: heads split 2-per-core (columns of Wq/Wk/Wv, rows of Wo). Each core
produces a partial y (transposed, bf16); the host sums the 8 partials in f32.

Per-core pipeline (v4):
  - Scores are computed transposed (scoresT [m, n]) so the PV contraction (m)
    lands on partitions; softmax denominators come from a ones-column in v;
    no max subtraction (scores ~ N(0,1), fp32 exp is safe).
  - The n axis runs in blocks [256, 512, 512, 512, 256]: narrow ends shrink
    the x-gated startup (first exp needs only x[:, :256]) and the
    serial tail after the last exp (norm + projection of a 256-wide block).
  - Within a block, PV(mt) is issued AT_LEAD=2 iterations behind scores(mt)
    so the exp never sits in the PE's in-order dependency chain; the Scalar
    engine streams exps back-to-back. (Deliberately NOT pipelined across
    block boundaries: fully saturating every engine measurably inflates all
    instruction durations ~20% — the small boundary pauses are cheaper.)
  - HBM reads run ~45-70 GB/s per DMA ring (~165 aggregate), making block 0
    inherently DMA-paced: ctx arrives as 16 [P,CK,128] pieces over the
    sync/gpsimd rings, consumed just-in-time by per-chunk kT emission and
    v emission inside block 0's loop. x streams on the scalar ring; later
    pieces are issued mid-loop (the scheduler hoists each issue to the
    moment its ring frees).
  - Block end: PV psum is evacuated to SBUF (copy), freeing its banks for
    the next block; recip -> gpsimd partition-broadcast -> mul runs off
    the critical path. The previous block's Wo projection and the next
    block's qT emission ride the PE slack of later iterations.
  - PSUM budget: score ping-pong 2x2 banks + PV 2 + aux(v/proj) 1 +
    emit(kT chunks/qT) 1 = 8 banks.
"""

import os
import sys

for _p in ("/opt/trn_rl_repo", "/root/.axon_site/_ro/trn_rl_repo"):
    if os.path.isdir(_p) and _p not in sys.path:
        sys.path.insert(0, _p)

import numpy as np
import ml_dtypes

import concourse.bass as bass
import concourse.mybir as mybir
import concourse.tile as tile
from concourse import bacc
from concourse.bass_utils import run_bass_kernel_spmd

P = 128
N_TOK = 2048  # n: query rows
M_TOK = 2048  # m: context rows
D = 1024
C = 768
HEADS = 16
DH = 64  # head dim
HPC = 2  # heads per core
SCALE = 8.0  # sqrt(DH)

NB = 512  # maximum n-block width (psum tile size)
BLOCKS = [(0, 512), (512, 512), (1024, 512), (1536, 512)]
DK = D // P  # 8 contraction chunks for x projections
CK = C // P  # 6 contraction chunks for ctx projections
MT = M_TOK // P  # 16 context chunks
AT_LEAD = 2  # PV trails scores by this many mt iterations

DTYPE_MODE = os.environ.get("CA_DTYPE", "bf16")


def _dtypes():
    if DTYPE_MODE == "bf16":
        return mybir.dt.bfloat16, ml_dtypes.bfloat16, mybir.dt.bfloat16
    if DTYPE_MODE == "f32r":
        return mybir.dt.float32r, np.float32, mybir.dt.float32r
    return mybir.dt.float32, np.float32, mybir.dt.float32


def _mm_cast(ap, mm_dt):
    return ap.bitcast(mm_dt) if ap.dtype != mm_dt else ap


def build_core_program():
    dt_store, _, dt_mm = _dtypes()
    f32 = mybir.dt.float32

    nc = bacc.Bacc("TRN2", target_bir_lowering=False, debug=False)

    xT = nc.declare_dram_parameter("xT", [D, N_TOK], dt_store, isOutput=False)
    ctxT = nc.declare_dram_parameter("ctxT", [C, M_TOK], dt_store, isOutput=False)
    wq = nc.declare_dram_parameter("wq", [D, P], dt_store, isOutput=False)
    wk = nc.declare_dram_parameter("wk", [C, P], dt_store, isOutput=False)
    wv = nc.declare_dram_parameter("wv", [C, P], dt_store, isOutput=False)
    wo = nc.declare_dram_parameter("wo", [P, D], dt_store, isOutput=False)
    yT = nc.declare_dram_parameter("yT", [D, N_TOK], dt_store, isOutput=True)

    with tile.TileContext(nc) as tc:
        with (
            tc.tile_pool(name="wts", bufs=1) as wts,
            tc.tile_pool(name="att", bufs=4) as att,
            tc.tile_pool(name="yout", bufs=6) as yout,
            tc.tile_pool(name="small", bufs=2) as small,
            tc.tile_pool(name="ps_sc", bufs=2, space="PSUM") as ps_sc,  # 2x2 banks
            tc.tile_pool(name="ps_pv", bufs=2, space="PSUM") as ps_pv,  # 2x1
            tc.tile_pool(name="ps_aux", bufs=1, space="PSUM") as ps_aux,  # 1
            tc.tile_pool(name="ps_emit", bufs=1, space="PSUM") as ps_emit,  # 1
        ):
            # ---- input DMA, phase 1: weights + the first-exp gates.
            # x's first 256 columns split into dk-halves on two rings. ----
            wk_sb = wts.tile([P, CK, P], dt_store)
            nc.sync.dma_start(wk_sb[:], wk.ap().rearrange("(p o) e -> p o e", o=CK))
            wq_sb = wts.tile([P, DK, P], dt_store)
            nc.scalar.dma_start(wq_sb[:], wq.ap().rearrange("(p o) e -> p o e", o=DK))
            wv_sb = wts.tile([P, CK, P], dt_store)
            nc.gpsimd.dma_start(wv_sb[:], wv.ap().rearrange("(p o) e -> p o e", o=CK))

            ctxT_sb = wts.tile([P, CK, M_TOK], dt_store)
            ctx_src = ctxT.ap().rearrange("(o p) m -> p o m", p=P)
            xT_sb = wts.tile([P, DK, N_TOK], dt_store)
            x_src = xT.ap().rearrange("(o p) m -> p o m", p=P)

            W0 = BLOCKS[0][1]
            nc.scalar.dma_start(xT_sb[:, :4, :W0], x_src[:, :4, :W0])
            nc.gpsimd.dma_start(xT_sb[:, 4:, :W0], x_src[:, 4:, :W0])

            # ACT exp-table preload: tiny dummy exp, after the early DMA
            # issues; the ~1.3us ACT_TABLE_LOAD hides under the input DMA
            warm = small.tile([1, 8], f32, tag="warm", bufs=1)
            nc.vector.memset(warm[:], 0.0)
            nc.scalar.activation(warm[:], warm[:], mybir.ActivationFunctionType.Exp)

            # ---- input DMA, phase 2: ctx as 16 need-ordered 128-column
            # pieces over sync/gpsimd so v(mt)/kT(mt) stall only on the
            # exact chunk they touch; wo + x[1280:1792] ride sync last
            # (their queue-blocking issues are benign there) ----
            def ctx_piece(mt, eng):
                sl = slice(mt * P, (mt + 1) * P)
                eng.dma_start(ctxT_sb[:, :, sl], ctx_src[:, :, sl])

            for mt in (0, 1, 2, 3, 5, 7, 9, 11):
                ctx_piece(mt, nc.sync)
            for mt in (4, 6, 8, 10, 12, 14):
                ctx_piece(mt, nc.gpsimd)
            wo_sb = wts.tile([P, D], dt_store)
            nc.sync.dma_start(wo_sb[:], wo.ap())

            # ---- persistent intermediates ----
            kT_sb = wts.tile([P, M_TOK], dt_store)  # [dh(2 heads), m]
            qT_sb = wts.tile([P, N_TOK], dt_store)  # [dq(2 heads), n]
            # v_aug layout [m, mt, 128]: col 0 = ones (softmax sums land on
            # PSUM partition 0), cols 64..127 = v values (normalize reads a
            # window that doesn't straddle the 64-partition boundary)
            VW = 128
            vA_sb = wts.tile([P, MT, VW], dt_store)
            vB_sb = wts.tile([P, MT, VW], dt_store)
            oT_sb = wts.tile([P, N_TOK], dt_store)  # attn out^T, both heads

            def _memset(ap, val):
                if ap.dtype == mybir.dt.float32r:
                    ap = ap.bitcast(f32)
                nc.vector.memset(ap, val)

            _memset(vA_sb[:], 0.0)
            _memset(vB_sb[:], 0.0)
            _memset(vA_sb[:, :, 0:1], 1.0)
            _memset(vB_sb[:, :, 0:1], 1.0)

            def mm(out, lhsT, rhs, start, stop):
                nc.tensor.matmul(
                    out, _mm_cast(lhsT, dt_mm), _mm_cast(rhs, dt_mm),
                    start=start, stop=stop,
                )

            # ---- staged emissions through the 1-bank emit psum ----
            def emit_kT_chunk(mt):
                # kT for one 128-wide m-chunk: 6 accumulating matmuls + copy
                ps = ps_emit.tile([P, P], f32, tag="emit", name="ps_kc")
                msl = slice(mt * P, (mt + 1) * P)
                for ck in range(CK):
                    mm(ps, wk_sb[:, ck, :], ctxT_sb[:, ck, msl],
                       start=(ck == 0), stop=(ck == CK - 1))
                nc.vector.tensor_copy(kT_sb[:, msl], ps)

            emit_ps = {}

            def qT_step(j, lo, hi):
                # qT for block j (columns BLOCKS[j]), contraction chunks
                # [lo, hi) of DK, through the emit psum
                n0, w = BLOCKS[j]
                if lo == 0:
                    emit_ps[j] = ps_emit.tile(
                        [P, NB], f32, tag="emit", name=f"ps_q{j}"
                    )
                ps = emit_ps[j][:, :w]
                for c in range(lo, hi):
                    mm(ps, wq_sb[:, c, :], xT_sb[:, c, n0 : n0 + w],
                       start=(c == 0), stop=(c == DK - 1))
                if hi == DK:
                    nc.vector.tensor_copy(qT_sb[:, n0 : n0 + w], ps)

            def emit_v(mt):
                ps = ps_aux.tile([P, NB], f32, tag="aux", name="ps_v")[:, :P]
                for ck in range(CK):
                    mm(ps, ctxT_sb[:, ck, mt * P : (mt + 1) * P], wv_sb[:, ck, :],
                       start=(ck == 0), stop=(ck == CK - 1))
                nc.vector.tensor_copy(vA_sb[:, mt, 64 : 64 + DH], ps[:, :DH])
                nc.vector.tensor_copy(vB_sb[:, mt, 64 : 64 + DH], ps[:, DH:])

            def emit_proj(s, j):
                # one 128-row slab of yT for block j's columns
                n0, w = BLOCKS[j]
                ps = ps_aux.tile([P, NB], f32, tag="aux", name="ps_proj")[:, :w]
                mm(ps, wo_sb[:, s * P : (s + 1) * P], oT_sb[:, n0 : n0 + w],
                   start=True, stop=True)
                ys = yout.tile([P, NB], dt_store, tag="yout", name="ys")[:, :w]
                nc.vector.tensor_copy(ys, ps)
                nc.sync.dma_start(yT.ap()[s * P : (s + 1) * P, n0 : n0 + w], ys)

            # per-(nb, mt) extra PE work, sized to the ACT exp cadence
            extras = {}

            def add_extra(nb, mt, fn):
                extras.setdefault((nb, mt), []).append(fn)

            # block 0: JIT kT chunk mt+1 at iteration mt (chunk 0 is in the
            # prologue); v(mt) is emitted in the main loop body
            for mt in range(MT - 1):
                add_extra(0, mt, lambda mt=mt: emit_kT_chunk(mt + 1))
            # blocks 1-2 produce qT(nb+1) spread over mt 8..11
            for nbb in (1, 2):
                for i, (lo, hi) in enumerate(((0, 2), (2, 4), (4, 6), (6, 8))):
                    add_extra(nbb, 8 + i,
                              lambda j=nbb + 1, lo=lo, hi=hi: qT_step(j, lo, hi))
            # blocks 1-3 run the previous block's Wo projection at mt 6..13
            for nbb in (1, 2, 3):
                for s in range(8):
                    add_extra(nbb, 6 + s, lambda s=s, j=nbb - 1: emit_proj(s, j))
            # late transfers on the scalar ring (hoisted to ring-free
            # moments by the scheduler)
            def x_piece(lo, hi):
                return lambda: nc.scalar.dma_start(
                    xT_sb[:, :, lo:hi], x_src[:, :, lo:hi]
                )

            late_issues = {
                (0, 0): x_piece(512, 1024),
                (0, 7): lambda: ctx_piece(13, nc.scalar),
                (0, 10): lambda: ctx_piece(15, nc.scalar),
                (0, 12): x_piece(1024, 1536),
                (1, 2): x_piece(1536, 2048),
            }

            # ---- prologue: kT chunk 0, then qT(0) ----
            with nc.named_scope("prologue"):
                emit_kT_chunk(0)
                pq = ps_sc.tile([P, 2, NB], f32, tag="sc", name="pq")[:, 0, :W0]
                for dk in range(DK):
                    mm(pq, wq_sb[:, dk, :], xT_sb[:, dk, :W0],
                       start=(dk == 0), stop=(dk == DK - 1))
                nc.vector.tensor_copy(qT_sb[:, :W0], pq)

            # ---- attention blocks ----
            def emit_pv(pvA, pvB, at, j, w):
                st, sp = (j == 0), (j == MT - 1)
                mm(pvA[:, :w], vA_sb[:, j, :], at[:, 0, :w], start=st, stop=sp)
                mm(pvB[:, :w], vB_sb[:, j, :], at[:, 1, :w], start=st, stop=sp)

            for nb, (n0, w) in enumerate(BLOCKS):
                nsl = slice(n0, n0 + w)
                last = nb == len(BLOCKS) - 1
                with nc.named_scope(f"att{nb}"):
                    pvA = ps_pv.tile([P, NB], f32, tag="pv", name="pvA")
                    pvB = ps_pv.tile([P, NB], f32, tag="pv", name="pvB")
                    at_ring = {}
                    for mt in range(MT):
                        msl = slice(mt * P, (mt + 1) * P)
                        sc = ps_sc.tile([P, 2, NB], f32, tag="sc", name="sc")
                        mm(sc[:, 0, :w], kT_sb[0:DH, msl], qT_sb[0:DH, nsl],
                           start=True, stop=True)
                        mm(sc[:, 1, :w], kT_sb[DH:P, msl], qT_sb[DH:P, nsl],
                           start=True, stop=True)
                        at = att.tile([P, 2, NB], dt_store, tag="at", name="at")
                        nc.scalar.activation(
                            at[:, :, :w], sc[:, :, :w],
                            mybir.ActivationFunctionType.Exp,
                        )
                        at_ring[mt] = at
                        issue = late_issues.get((nb, mt))
                        if issue is not None:
                            issue()
                        if nb == 0:
                            emit_v(mt)
                        for fn in extras.get((nb, mt), ()):
                            fn()
                        j = mt - AT_LEAD
                        if j >= 0:
                            emit_pv(pvA, pvB, at_ring.pop(j), j, w)
                    for j in range(MT - AT_LEAD, MT):
                        emit_pv(pvA, pvB, at_ring.pop(j), j, w)
                    if nb == 0:
                        # qT(1) lump: the emit bank was busy with kT chunks
                        # all through block 0
                        qT_step(1, 0, DK)
                    if not last:
                        # evacuate PV psum (frees the banks for the next
                        # block), then normalize from SBUF off-path
                        evA = small.tile([P, NB], f32, tag="evac", name="evA",
                                         bufs=2)
                        nc.vector.tensor_copy(evA[:, :w], pvA[:, :w])
                        evB = small.tile([P, NB], f32, tag="evac", name="evB",
                                         bufs=2)
                        nc.vector.tensor_copy(evB[:, :w], pvB[:, :w])
                        for h, ev in ((0, evA), (1, evB)):
                            rcf = small.tile([1, NB], f32, tag="recip",
                                             name="rcf")
                            nc.vector.reciprocal_approx_fast(
                                rcf[:, :w], ev[0:1, :w]
                            )
                            bcs = small.tile([P, NB], f32, tag="bcast",
                                             name="bcs")
                            nc.gpsimd.partition_broadcast(bcs[:, :w], rcf[:, :w])
                            nc.vector.tensor_mul(
                                oT_sb[h * DH : (h + 1) * DH, nsl],
                                ev[64 : 64 + DH, :w], bcs[64:P, :w],
                            )

            # ---- tail: the last (256-wide) block normalizes straight from
            # PSUM (the mul mixes PSUM + SBUF inputs, so a 64-row broadcast
            # works), then 8 projection slabs spread over all four psum
            # tags; copies alternate DVE/Scalar ----
            with nc.named_scope("tail"):
                nL, wL = BLOCKS[-1]
                nslL = slice(nL, nL + wL)
                for h, pv in ((0, pvA), (1, pvB)):
                    rcf = small.tile([1, NB], f32, tag="recip", name="rcf")
                    nc.vector.reciprocal_approx_fast(rcf[:, :wL], pv[0:1, :wL])
                    bcs = small.tile([DH, NB], f32, tag="bcast_t", name="bcs",
                                     bufs=2)
                    nc.gpsimd.partition_broadcast(bcs[:, :wL], rcf[:, :wL])
                    nc.vector.tensor_mul(
                        oT_sb[h * DH : (h + 1) * DH, nslL],
                        pv[64 : 64 + DH, :wL], bcs[:, :wL],
                    )
                for s in range(8):
                    ps = ps_sc.tile([P, 2, NB], f32, tag="sc",
                                    name="ps_ty")[:, 0, :wL]
                    mm(ps, wo_sb[:, s * P : (s + 1) * P], oT_sb[:, nslL],
                       start=True, stop=True)
                    ys = yout.tile([P, NB], dt_store, tag="yout",
                                   name="ys")[:, :wL]
                    if s % 2 == 0:
                        nc.vector.tensor_copy(ys, ps)
                    else:
                        nc.scalar.copy(ys, ps)
                    eng = nc.sync if s % 2 == 0 else nc.gpsimd
                    eng.dma_start(yT.ap()[s * P : (s + 1) * P, nslL], ys)

    nc.compile()
    return nc


_NC_CACHE = {}


def _get_nc():
    key = DTYPE_MODE
    if key not in _NC_CACHE:
        _NC_CACHE[key] = build_core_program()
    return _NC_CACHE[key]


def _shuffle_w(w):
    # [o*P + p, e] -> [p*o_n + o, e] so each SBUF partition's rows are
    # contiguous in DRAM (single contiguous DMA into a [P, o, e] tile)
    o_n = w.shape[0] // P
    return np.ascontiguousarray(
        w.reshape(o_n, P, w.shape[1]).transpose(1, 0, 2).reshape(w.shape)
    )


def _prep_in_maps(x, ctx, Wq, Wk, Wv, Wo):
    _, np_dt, _ = _dtypes()
    xT = np.ascontiguousarray(x.T).astype(np_dt)
    ctxT = np.ascontiguousarray(ctx.T).astype(np_dt)
    Wq_s = (Wq / SCALE).astype(np.float32)
    in_maps = []
    for cc in range(8):
        csl = slice(cc * P, (cc + 1) * P)
        in_maps.append(
            {
                "xT": xT,
                "ctxT": ctxT,
                "wq": _shuffle_w(np.ascontiguousarray(Wq_s[:, csl])).astype(np_dt),
                "wk": _shuffle_w(np.ascontiguousarray(Wk[:, csl])).astype(np_dt),
                "wv": _shuffle_w(np.ascontiguousarray(Wv[:, csl])).astype(np_dt),
                "wo": np.ascontiguousarray(Wo[csl, :]).astype(np_dt),
            }
        )
    return in_maps


def run(x, ctx, Wq, Wk, Wv, Wo, trace=False):
    nc = _get_nc()
    in_maps = _prep_in_maps(x, ctx, Wq, Wk, Wv, Wo)
    res = run_bass_kernel_spmd(nc, in_maps, core_ids=list(range(8)), trace=trace)
    acc = np.zeros((D, N_TOK), np.float32)
    for r in res.results:
        acc += np.asarray(r["yT"], dtype=np.float32)
    return np.ascontiguousarray(acc.T), res


def kernel(x, ctx, Wq, Wk, Wv, Wo):
    x = np.asarray(x, dtype=np.float32)
    ctx = np.asarray(ctx, dtype=np.float32)
    Wq = np.asarray(Wq, dtype=np.float32)
    Wk = np.asarray(Wk, dtype=np.float32)
    Wv = np.asarray(Wv, dtype=np.float32)
    Wo = np.asarray(Wo, dtype=np.float32)
    y, _ = run(x, ctx, Wq, Wk, Wv, Wo, trace=False)
    return y

